# revision 1
# baseline (speedup 1.0000x reference)
"""PolyMatchingLoss Trainium2 kernel.

Reference computation (B=128, P=1024, C=2):
    dis[b, i] = mean_j sum_c smooth_l1(pred[b,j,c] - gt[b,(i+j)%P,c])
    out = mean_b min_i dis[b, i]

Strategy:
  - Pure data parallel over batch: 16 batches per core x 8 cores.
  - Per (b, shift-block qi): one fused custom DVE instruction computes
    2*smooth_l1(W - P) elementwise over a [128 shifts, 2048 (j,c)] tile
    and reduces (sum) along the free axis into a [128,1] accumulator
    column.  smooth_l1 via m*(2t-m) = 2*huber, t=|d|, m=min(t,1).
  - The gt operand uses the "staircase" identity: W[x, y] = gtflat2[2x+y]
    (c-interleaved, cyclically duplicated gt).  One [128, 3840] window
    tile per (b) serves all 8 shift blocks as free-axis offset slices.
  - pred is broadcast along partitions (host-replicated).
  - min over shifts + mean over batch on host (tiny).
"""

from operator import add as _operator_add

import numpy as np

from concourse import mybir
from concourse import bass, bass_utils
from concourse.tile import TileContext
import concourse.dve_ops as _dve_ops
from concourse.dve_ops import DveOp
from concourse.dve_spec import Spec, Src0, Src1, Zero, One, maxx, minn

# ---------------------------------------------------------------------------
# Workaround: this toolchain's walrus allows at most ONE sync wait per
# instruction; Tile emits 2+.  Split extras onto EventSemaphore carrier
# instructions inserted just before the offending instruction.
# ---------------------------------------------------------------------------
def _split_multi_waits(nc) -> int:
    n = 0
    for fn in nc.m.functions:
        for bb in fn.blocks:
            out = []
            for inst in bb.instructions:
                si = inst.sync_info
                if si is not None and si.on_wait and len(si.on_wait) > 1:
                    for k, w in enumerate(si.on_wait[:-1]):
                        out.append(
                            mybir.InstEventSemaphore(
                                name=f"{inst.name}_wsplit{k}",
                                opcode="EventSemaphore",
                                engine=inst.engine,
                                ins=[],
                                outs=[],
                                sync_info=mybir.SyncInfo(on_wait=[w], on_update=[]),
                            )
                        )
                        n += 1
                    si.on_wait = [si.on_wait[-1]]
                out.append(inst)
            bb.instructions = out
    return n


B = 128
PNUM = 1024
C = 2
NCORES = 8
BL = B // NCORES  # batches per core
FD = PNUM * C  # 2048 free elements per tile
WW = FD + 256 * 7  # 3840 window width


# --------------------------------------------------------------------------
# Custom DVE op: out = m*(2t - m) with t=|in0-in1|, m=min(t,1)  (= 2*huber)
#                accum_out = sum over free axis
# --------------------------------------------------------------------------
def _huber_ref(in0, in1, s0, s1, imm2):
    dd = in0.astype(np.float32) - in1.astype(np.float32)
    tt = np.abs(dd)
    mm = np.minimum(tt, 1.0)
    bb = (mm * (2.0 * tt - mm)).astype(np.float32)
    return bb, bb.reshape(bb.shape[0], -1).sum(axis=-1, keepdims=True)


def _make_huber_op() -> DveOp:
    d = Src0 - Src1
    nd = Src1 - Src0
    t = maxx(d, nd)
    m = minn(t, One)
    v = t - m
    w = t + v
    body = m * w
    return DveOp(
        "TENSOR_HUBER2_REDUCE",
        Spec(body=body, accum=_operator_add, accum_init=Zero, reference=_huber_ref),
        subdim=False,
        uops_sha={"v3": "e8f6160a1f1db788", "v4": "8b26f7daea78cb80"},
    )


def _register_op(op: DveOp) -> None:
    if op.name in _dve_ops._SUB_OPCODE_FOR_NAME:
        return
    _dve_ops.OPS.append(op)
    _dve_ops._SUB_OPCODE_FOR_NAME[op.name] = (
        _dve_ops._CUSTOM_DVE_ROW_BASE + len(_dve_ops.OPS) - 1
    )
    _dve_ops.CUSTOM_DVE_SPECS[op.name] = op.spec
    assert _dve_ops._SUB_OPCODE_FOR_NAME[op.name] < 0x20


HUBER_OP = _make_huber_op()
_register_op(HUBER_OP)


# --------------------------------------------------------------------------
# Bass program (SPMD, one program for all 8 cores)
# --------------------------------------------------------------------------
_dt = mybir.dt
_program_cache = {}


NA = 0  # ACT+PE hybrid path disabled: measured no gain over the pure DVE path
WB = 1024 + 128 * 7  # 1920: layout-B window width per coordinate


def _build_program(reps: int = 1, na: int = NA):
    nc = bass.Bass()
    gtw = nc.declare_dram_parameter("gtw", [BL, 2 * FD], _dt.float32, isOutput=False)
    prep = nc.declare_dram_parameter(
        "prep", [BL, 128, FD], _dt.float32, isOutput=False
    )
    gtsep = nc.declare_dram_parameter(
        "gtsep", [BL, 2, 2048], _dt.float32, isOutput=False
    )
    prednegt = nc.declare_dram_parameter(
        "prednegt", [BL, 2, 128, 8], _dt.float32, isOutput=False
    )
    acc_out = nc.declare_dram_parameter(
        "acc", [128, BL * 8], _dt.float32, isOutput=True
    )
    acc2_out = nc.declare_dram_parameter(
        "acc2", [max(na, 1), 1024], _dt.float32, isOutput=True
    )

    AF = mybir.ActivationFunctionType
    with TileContext(nc) as tc:
        with (
            tc.tile_pool(name="w", bufs=3) as wpool,
            tc.tile_pool(name="p", bufs=3) as ppool,
            tc.tile_pool(name="s", bufs=2) as spool,
            tc.tile_pool(name="a", bufs=1) as apool,
            tc.tile_pool(name="wb", bufs=2) as wbpool,
            tc.tile_pool(name="pc", bufs=2) as pcpool,
            tc.tile_pool(name="act", bufs=2) as actpool,
            tc.tile_pool(name="one", bufs=1) as onepool,
            tc.tile_pool(name="ds", bufs=2) as dspool,
            tc.tile_pool(name="ps", bufs=2, space="PSUM") as pspool,
        ):
            acc = apool.tile([128, BL * 8], _dt.float32)
            nc.vector.memset(acc[:], 0.0)
            onep = onepool.tile([128, 1], _dt.float32, tag="onep")
            onen = onepool.tile([128, 1], _dt.float32, tag="onen")
            nc.vector.memset(onep[:], 1.0)
            nc.vector.memset(onen[:], -1.0)
            dsz = dspool.tile([1, 1024], _dt.float32, tag="dsz")
            nc.vector.memset(dsz[:], 0.0)
            if na < 1:
                nc.sync.dma_start(out=acc2_out[0], in_=dsz[0:1, :])
            for _rep in range(reps):
                for b in range(BL - na):
                    w = wpool.tile([128, WW], _dt.float32)
                    # staircase window: row x = gtflat2[b, 2x : 2x + WW]
                    nc.sync.dma_start(
                        out=w[:], in_=bass.AP(gtw, b * 2 * FD, [[2, 128], [1, WW]])
                    )
                    p = ppool.tile([128, FD], _dt.float32)
                    nc.sync.dma_start(out=p[:], in_=prep[b])
                    for qi in range(8):
                        scr = spool.tile([128, FD], _dt.float32)
                        col = b * 8 + qi
                        nc.vector._custom_dve(
                            HUBER_OP,
                            out=scr[:],
                            in0=w[:, 256 * qi : 256 * qi + FD],
                            in1=p[:],
                            accum_out=acc[:, col : col + 1],
                        )
                for bi in range(na):
                    b = BL - na + bi
                    # layout B: partition = point-in-block u, free = shift i.
                    # DMAs issue from the ACT engine's sequencer so they are
                    # not head-of-line blocked behind the DVE path's W/P DMA
                    # slot-reuse waits on the sync engine.
                    wb = wbpool.tile([128, 2 * WB], _dt.float32)
                    for c in (0, 1):
                        nc.scalar.dma_start(
                            out=wb[:, c * WB : c * WB + WB],
                            in_=bass.AP(gtsep, (b * 2 + c) * 2048, [[1, 128], [1, WB]]),
                        )
                    pcol = pcpool.tile([128, 16], _dt.float32)
                    for c in (0, 1):
                        nc.scalar.dma_start(
                            out=pcol[:, c * 8 : c * 8 + 8], in_=prednegt[b, c]
                        )
                    psA = pspool.tile([1, 512], _dt.float32, tag="psA")
                    psB = pspool.tile([1, 512], _dt.float32, tag="psB")
                    for c in (0, 1):
                        for qi in range(8):
                            src = wb[:, c * WB + 128 * qi : c * WB + 128 * qi + 1024]
                            bias = pcol[:, c * 8 + qi : c * 8 + qi + 1]
                            tt = actpool.tile([128, 1024], _dt.float32, tag="tt")
                            nc.scalar.activation(tt[:], src, AF.Abs, bias=bias)
                            rr = actpool.tile([128, 1024], _dt.float32, tag="rr")
                            nc.scalar.activation(rr[:], tt[:], AF.Relu, bias=onen[:, 0:1])
                            t2 = actpool.tile([128, 1024], _dt.float32, tag="t2")
                            nc.scalar.activation(t2[:], tt[:], AF.Square)
                            r2 = actpool.tile([128, 1024], _dt.float32, tag="r2")
                            nc.scalar.activation(r2[:], rr[:], AF.Square)
                            first = c == 0 and qi == 0
                            last = c == 1 and qi == 7
                            for h, ps in ((0, psA), (1, psB)):
                                sl = slice(512 * h, 512 * h + 512)
                                nc.tensor.matmul(
                                    ps[0:1, :], onep[:, 0:1], t2[:, sl],
                                    start=first, stop=False,
                                )
                                nc.tensor.matmul(
                                    ps[0:1, :], onen[:, 0:1], r2[:, sl],
                                    start=False, stop=last,
                                )
                    dsb = dspool.tile([1, 1024], _dt.float32)
                    nc.vector.tensor_copy(dsb[0:1, 0:512], psA[0:1, :])
                    nc.vector.tensor_copy(dsb[0:1, 512:1024], psB[0:1, :])
                    nc.scalar.dma_start(out=acc2_out[bi], in_=dsb[0:1, :])
            nc.sync.dma_start(out=acc_out[:], in_=acc[:])
    _split_multi_waits(nc)
    # Raw Bass (unlike Bacc.compile) never runs this pass; without it the
    # custom-DVE InstISA subclasses serialize with empty .instr bytes and
    # walrus fails with "ISA wrong length".
    mybir.codegen_inst_isa_subclasses(nc)
    return nc


def _get_program():
    if "nc" not in _program_cache:
        _program_cache["nc"] = _build_program()
    return _program_cache["nc"]


# --------------------------------------------------------------------------
# Host wrapper
# --------------------------------------------------------------------------
def _make_in_maps(pred: np.ndarray, gt: np.ndarray):
    pred = np.ascontiguousarray(pred, dtype=np.float32)
    gt = np.ascontiguousarray(gt, dtype=np.float32)
    in_maps = []
    for c in range(NCORES):
        sl = slice(c * BL, (c + 1) * BL)
        gtc = gt[sl]  # [BL, P, C]
        gtdupc = np.concatenate([gtc, gtc], axis=1)  # [BL, 2P, C]
        gtdup = gtdupc.reshape(BL, 2 * FD)
        predc = pred[sl].reshape(BL, 1, FD)
        prep = np.ascontiguousarray(np.broadcast_to(predc, (BL, 128, FD)))
        # layout-B side: c-separated windows + negated pred columns
        gtsep = np.ascontiguousarray(gtdupc.transpose(0, 2, 1))  # [BL, 2, 2048]
        prednegt = np.ascontiguousarray(
            (-pred[sl]).reshape(BL, 8, 128, 2).transpose(0, 3, 2, 1)
        )  # [BL, 2, 128, 8] = -pred[b, 128q+u, c]
        in_maps.append(
            {"gtw": gtdup, "prep": prep, "gtsep": gtsep, "prednegt": prednegt}
        )
    return in_maps


def _finish(results, na: int = NA) -> np.float32:
    mins = []
    for c in range(NCORES):
        acc = np.asarray(results[c]["acc"], dtype=np.float32)  # [128, BL*8]
        acc = acc.reshape(128, BL, 8)  # [i_local, b, qi]
        dis = acc.transpose(1, 2, 0).reshape(BL, PNUM) / (2.0 * PNUM)
        mins.append(dis[: BL - na].min(axis=1))
        if na > 0:
            acc2 = np.asarray(results[c]["acc2"], dtype=np.float32)  # [na, 1024]
            mins.append((acc2 / (2.0 * PNUM)).min(axis=1))
    return np.asarray(np.mean(np.concatenate(mins)), dtype=np.float32)


def kernel(pred: np.ndarray, gt: np.ndarray) -> np.ndarray:
    nc = _get_program()
    in_maps = _make_in_maps(pred, gt)
    res = bass_utils.run_bass_kernel_spmd(nc, in_maps, list(range(NCORES)))
    return _finish(res.results)


# Exposed for test.py: run with tracing and return (value, BassKernelResults)
def kernel_traced(pred: np.ndarray, gt: np.ndarray, **kw):
    nc = _get_program()
    in_maps = _make_in_maps(pred, gt)
    res = bass_utils.run_bass_kernel_spmd(nc, in_maps, list(range(NCORES)), **kw)
    return _finish(res.results), res



# revision 6
# speedup vs baseline: 1.6283x; 1.6283x over previous
"""PolyMatchingLoss Trainium2 kernel.

Reference computation (B=128, P=1024, C=2):
    dis[b, i] = mean_j sum_c smooth_l1(pred[b,j,c] - gt[b,(i+j)%P,c])
    out = mean_b min_i dis[b, i]

Strategy (v2): two concurrent per-core lanes over the 16 local batches.

  Lane D (DVE, ND batches): one fused custom DVE instruction per
    (b, shift-block) computes 2*smooth_l1(W - P) elementwise over a
    [128 shifts, 2048 (j,c)] tile and sum-reduces along the free axis
    into a [128,1] accumulator column.  smooth_l1 via m*(2t-m), t=|d|,
    m=min(t,1).  The gt operand uses the staircase identity
    W[x, y] = gtflat2[2x+y]; pred is host-replicated to 128 partitions.

  Lane C (ACT+PE, NC batches): uses 2f(d) = d^2 - relu(|d|-1)^2.
    Layout: partition = point-in-block u, free = shift i.
    - Sum_j d^2 = (Sum p^2 + Sum g^2) - 2 corr[i]; the constant is added
      on the host, corr[i] is computed by TensorE as 8 accumulating
      matmuls per c whose stationary operand is the -2*pred block column
      and whose moving operand is the SAME gt window tile the ACT passes
      read.
    - The correction Sum_j relu(|d|-1)^2 runs on ScalarE in 3 passes
      (Abs with per-partition -pred bias; one big Relu(x-1); one big
      Square) and is reduced over partitions by TensorE with a -1s
      stationary column.
    - All matmuls of all NC batches accumulate into one stacked PSUM
      pair [8, 512]x2 via one-hot stationary columns (col = local batch
      index), so PSUM is copied out exactly once per rep.

  min over shifts + mean over batch on host (tiny).
"""

from operator import add as _operator_add

import numpy as np

from concourse import mybir
from concourse import bass, bass_utils
from concourse.tile import TileContext
import concourse.dve_ops as _dve_ops
from concourse.dve_ops import DveOp
from concourse.dve_spec import Spec, Src0, Src1, Zero, One, maxx, minn

# ---------------------------------------------------------------------------
# Workaround: this toolchain's walrus allows at most ONE sync wait per
# instruction; Tile emits 2+.  Split extras onto EventSemaphore carrier
# instructions inserted just before the offending instruction.
# ---------------------------------------------------------------------------
def _split_multi_waits(nc) -> int:
    n = 0
    for fn in nc.m.functions:
        for bb in fn.blocks:
            out = []
            for inst in bb.instructions:
                si = inst.sync_info
                if si is not None and si.on_wait and len(si.on_wait) > 1:
                    for k, w in enumerate(si.on_wait[:-1]):
                        out.append(
                            mybir.InstEventSemaphore(
                                name=f"{inst.name}_wsplit{k}",
                                opcode="EventSemaphore",
                                engine=inst.engine,
                                ins=[],
                                outs=[],
                                sync_info=mybir.SyncInfo(on_wait=[w], on_update=[]),
                            )
                        )
                        n += 1
                    si.on_wait = [si.on_wait[-1]]
                out.append(inst)
            bb.instructions = out
    return n


B = 128
PNUM = 1024
C = 2
NCORES = 8
BL = B // NCORES  # batches per core
FD = PNUM * C  # 2048 free elements per lane-D tile
WW = FD + 256 * 7  # 3840 lane-D window width
WB = 1024 + 128 * 7  # 1920 lane-C per-coordinate window width

ND = 11  # lane-D (DVE) batches per core
NC = BL - ND  # lane-C (ACT+PE) batches per core


# --------------------------------------------------------------------------
# Custom DVE op: out = m*(2t - m) with t=|in0-in1|, m=min(t,1)  (= 2*huber)
#                accum_out = sum over free axis
# --------------------------------------------------------------------------
def _huber_ref(in0, in1, s0, s1, imm2):
    dd = in0.astype(np.float32) - in1.astype(np.float32)
    tt = np.abs(dd)
    mm = np.minimum(tt, 1.0)
    bb = (mm * (2.0 * tt - mm)).astype(np.float32)
    return bb, bb.reshape(bb.shape[0], -1).sum(axis=-1, keepdims=True)


def _make_huber_op() -> DveOp:
    d = Src0 - Src1
    nd = Src1 - Src0
    t = maxx(d, nd)
    m = minn(t, One)
    v = t - m
    w = t + v
    body = m * w
    return DveOp(
        "TENSOR_HUBER2_REDUCE",
        Spec(body=body, accum=_operator_add, accum_init=Zero, reference=_huber_ref),
        subdim=False,
        uops_sha={"v3": "e8f6160a1f1db788", "v4": "8b26f7daea78cb80"},
    )


def _register_op(op: DveOp) -> None:
    if op.name in _dve_ops._SUB_OPCODE_FOR_NAME:
        return
    _dve_ops.OPS.append(op)
    _dve_ops._SUB_OPCODE_FOR_NAME[op.name] = (
        _dve_ops._CUSTOM_DVE_ROW_BASE + len(_dve_ops.OPS) - 1
    )
    _dve_ops.CUSTOM_DVE_SPECS[op.name] = op.spec
    assert _dve_ops._SUB_OPCODE_FOR_NAME[op.name] < 0x20


HUBER_OP = _make_huber_op()
_register_op(HUBER_OP)


# --------------------------------------------------------------------------
# Bass program (SPMD, one program for all 8 cores)
# --------------------------------------------------------------------------
_dt = mybir.dt
_program_cache = {}


def _build_program(reps: int = 1):
    nc = bass.Bass()
    AF = mybir.ActivationFunctionType

    # lane D inputs
    gtw = nc.declare_dram_parameter("gtw", [ND, 2 * FD], _dt.float32, isOutput=False)
    prep = nc.declare_dram_parameter(
        "prep", [ND, 128, FD], _dt.float32, isOutput=False
    )
    # lane C inputs
    gtsepb = nc.declare_dram_parameter(
        "gtsepb", [NC, 2, 2048], _dt.bfloat16, isOutput=False
    )
    pcolc = nc.declare_dram_parameter(
        "pcolc", [NC, 2, 128, 8], _dt.float32, isOutput=False
    )
    statp = nc.declare_dram_parameter(
        "statp", [128, NC * 16 * 8], _dt.bfloat16, isOutput=False
    )
    stato = nc.declare_dram_parameter(
        "stato", [128, NC * 8], _dt.bfloat16, isOutput=False
    )
    # outputs
    acc_out = nc.declare_dram_parameter(
        "acc", [128, ND * 8], _dt.float32, isOutput=True
    )
    accc_out = nc.declare_dram_parameter("accc", [8, 1024], _dt.float32, isOutput=True)

    with TileContext(nc) as tc:
        with (
            tc.tile_pool(name="w", bufs=2) as wpool,
            tc.tile_pool(name="p", bufs=2) as ppool,
            tc.tile_pool(name="s", bufs=2) as spool,
            tc.tile_pool(name="a", bufs=1) as apool,
            tc.tile_pool(name="wb", bufs=2) as wbpool,
            tc.tile_pool(name="pc", bufs=2) as pcpool,
            tc.tile_pool(name="act", bufs=2) as actpool,
            tc.tile_pool(name="st", bufs=1) as stpool,
            tc.tile_pool(name="ac2", bufs=1) as ac2pool,
            tc.tile_pool(name="ps", bufs=1, space="PSUM") as pspool,
        ):
            acc = apool.tile([128, ND * 8], _dt.float32)
            nc.vector.memset(acc[:], 0.0)
            statpt = stpool.tile([128, NC * 16 * 8], _dt.bfloat16, tag="statpt")
            nc.sync.dma_start(out=statpt[:], in_=statp[:])
            statot = stpool.tile([128, NC * 8], _dt.bfloat16, tag="statot")
            nc.sync.dma_start(out=statot[:], in_=stato[:])
            onen = stpool.tile([128, 1], _dt.float32, tag="onen")
            nc.vector.memset(onen[:], -1.0)

            for _rep in range(reps):
                psA = pspool.tile([8, 512], _dt.float32, tag="psA")
                psB = pspool.tile([8, 512], _dt.float32, tag="psB")

                # ---------------- lane C (ACT + PE) ----------------
                nmm = 0  # matmul counter for start/stop flags
                NMM_TOTAL = NC * 2 * 8 * 2 * 2
                for bi in range(NC):
                    wb = wbpool.tile([128, 2 * WB], _dt.bfloat16)
                    for c in (0, 1):
                        # staircase: row u = gtsepb[bi, c, u : u + WB]
                        nc.scalar.dma_start(
                            out=wb[:, c * WB : (c + 1) * WB],
                            in_=bass.AP(
                                gtsepb, (bi * 2 + c) * 2048, [[1, 128], [1, WB]]
                            ),
                        )
                    pcol = pcpool.tile([128, 16], _dt.float32)
                    for c in (0, 1):
                        nc.scalar.dma_start(
                            out=pcol[:, c * 8 : (c + 1) * 8], in_=pcolc[bi, c]
                        )
                    for c in (0, 1):
                        # corr matmuls: -2*corr into psum rows [bi]
                        for q in range(8):
                            scol = ((bi * 2 + c) * 8 + q) * 8
                            for h, ps in ((0, psA), (1, psB)):
                                nc.tensor.matmul(
                                    ps[:, :],
                                    statpt[:, scol : scol + 8],
                                    wb[:, c * WB + 128 * q + 512 * h :][:, 0:512],
                                    start=(nmm == 0),
                                    stop=(nmm == NMM_TOTAL - 1),
                                )
                                nmm += 1
                        # ACT passes
                        tt = actpool.tile([128, 8192], _dt.bfloat16, tag="tt")
                        for q in range(8):
                            nc.scalar.activation(
                                tt[:, 1024 * q : 1024 * (q + 1)],
                                wb[:, c * WB + 128 * q :][:, 0:1024],
                                AF.Abs,
                                bias=pcol[:, c * 8 + q : c * 8 + q + 1],
                            )
                        rr = actpool.tile([128, 8192], _dt.bfloat16, tag="rr")
                        nc.scalar.activation(rr[:], tt[:], AF.Relu, bias=onen[:, 0:1])
                        r2 = actpool.tile([128, 8192], _dt.bfloat16, tag="r2")
                        nc.scalar.activation(r2[:], rr[:], AF.Square)
                        # reduction matmuls: -sum_u r2 into psum rows [bi]
                        for q in range(8):
                            for h, ps in ((0, psA), (1, psB)):
                                nc.tensor.matmul(
                                    ps[:, :],
                                    statot[:, bi * 8 : bi * 8 + 8],
                                    r2[:, 1024 * q + 512 * h :][:, 0:512],
                                    start=(nmm == 0),
                                    stop=(nmm == NMM_TOTAL - 1),
                                )
                                nmm += 1

                # ---------------- lane D (DVE) ----------------
                for b in range(ND):
                    w = wpool.tile([128, WW], _dt.float32)
                    # staircase window: row x = gtflat2[b, 2x : 2x + WW]
                    nc.sync.dma_start(
                        out=w[:], in_=bass.AP(gtw, b * 2 * FD, [[2, 128], [1, WW]])
                    )
                    p = ppool.tile([128, FD], _dt.float32)
                    nc.sync.dma_start(out=p[:], in_=prep[b])
                    for qi in range(8):
                        scr = spool.tile([128, FD], _dt.float32)
                        col = b * 8 + qi
                        nc.vector._custom_dve(
                            HUBER_OP,
                            out=scr[:],
                            in0=w[:, 256 * qi : 256 * qi + FD],
                            in1=p[:],
                            accum_out=acc[:, col : col + 1],
                        )

                # psum -> sbuf -> dram for lane C
                accc = ac2pool.tile([8, 1024], _dt.float32)
                nc.vector.tensor_copy(accc[:, 0:512], psA[:])
                nc.vector.tensor_copy(accc[:, 512:1024], psB[:])
                nc.scalar.dma_start(out=accc_out[:], in_=accc[:])

            nc.sync.dma_start(out=acc_out[:], in_=acc[:])
    _split_multi_waits(nc)
    # Raw Bass (unlike Bacc.compile) never runs this pass; without it the
    # custom-DVE InstISA subclasses serialize with empty .instr bytes and
    # walrus fails with "ISA wrong length".
    mybir.codegen_inst_isa_subclasses(nc)
    return nc


def _get_program():
    if "nc" not in _program_cache:
        _program_cache["nc"] = _build_program()
    return _program_cache["nc"]


# --------------------------------------------------------------------------
# Host wrapper
# --------------------------------------------------------------------------
def _make_in_maps(pred: np.ndarray, gt: np.ndarray):
    pred = np.ascontiguousarray(pred, dtype=np.float32)
    gt = np.ascontiguousarray(gt, dtype=np.float32)
    in_maps = []
    for core in range(NCORES):
        sl = slice(core * BL, (core + 1) * BL)
        gtc = gt[sl]  # [BL, P, C]
        predc = pred[sl]  # [BL, P, C]
        gtdupc = np.concatenate([gtc, gtc], axis=1)  # [BL, 2P, C]
        # lane D
        gtdup = gtdupc[:ND].reshape(ND, 2 * FD)
        prepc = np.ascontiguousarray(
            np.broadcast_to(predc[:ND].reshape(ND, 1, FD), (ND, 128, FD))
        )
        # lane C
        gtsepb = np.ascontiguousarray(
            gtdupc[ND:].transpose(0, 2, 1)
        )  # [NC, 2, 2048] f32
        pcolc = np.ascontiguousarray(
            (-predc[ND:]).reshape(NC, 8, 128, 2).transpose(0, 3, 2, 1)
        )  # [NC, 2, 128, 8] = -pred[b, 128q+u, c]
        # stationary tiles
        pblk = predc[ND:].reshape(NC, 8, 128, 2).transpose(2, 0, 3, 1)  # [128,NC,2,8]
        statp = np.zeros((128, NC, 2, 8, 8), dtype=np.float32)
        for bi in range(NC):
            statp[:, bi, :, :, bi] = -2.0 * pblk[:, bi, :, :]
        statp = statp.reshape(128, NC * 16 * 8)
        stato = np.zeros((128, NC, 8), dtype=np.float32)
        for bi in range(NC):
            stato[:, bi, bi] = -1.0
        stato = stato.reshape(128, NC * 8)
        in_maps.append(
            {
                "gtw": gtdup,
                "prep": prepc,
                "gtsepb": _to_bf16(gtsepb),
                "pcolc": pcolc,
                "statp": _to_bf16(statp),
                "stato": _to_bf16(stato),
            }
        )
    return in_maps


def _to_bf16(a: np.ndarray) -> np.ndarray:
    import ml_dtypes

    return a.astype(ml_dtypes.bfloat16)


def _finish(results, pred: np.ndarray, gt: np.ndarray) -> np.float32:
    pred = np.asarray(pred, dtype=np.float64)
    gt = np.asarray(gt, dtype=np.float64)
    mins = []
    for core in range(NCORES):
        sl = slice(core * BL, (core + 1) * BL)
        # lane D
        acc = np.asarray(results[core]["acc"], dtype=np.float32)  # [128, ND*8]
        acc = acc.reshape(128, ND, 8)  # [i_local, b, qi]
        dis = acc.transpose(1, 2, 0).reshape(ND, PNUM) / (2.0 * PNUM)
        mins.append(dis.min(axis=1))
        # lane C: 2P*dis = qc + psum  (psum = -2corr - sum rsq)
        accc = np.asarray(results[core]["accc"], dtype=np.float64)[:NC]  # [NC,1024]
        pc = pred[sl][ND:]
        gc = gt[sl][ND:]
        qc = (pc * pc).sum(axis=(1, 2)) + (gc * gc).sum(axis=(1, 2))  # [NC]
        disc = (qc[:, None] + accc) / (2.0 * PNUM)
        mins.append(disc.min(axis=1).astype(np.float32))
    return np.asarray(np.mean(np.concatenate(mins)), dtype=np.float32)


def kernel(pred: np.ndarray, gt: np.ndarray) -> np.ndarray:
    nc = _get_program()
    in_maps = _make_in_maps(pred, gt)
    res = bass_utils.run_bass_kernel_spmd(nc, in_maps, list(range(NCORES)))
    return _finish(res.results, pred, gt)


# Exposed for test.py: run with tracing and return (value, BassKernelResults)
def kernel_traced(pred: np.ndarray, gt: np.ndarray, **kw):
    nc = _get_program()
    in_maps = _make_in_maps(pred, gt)
    res = bass_utils.run_bass_kernel_spmd(nc, in_maps, list(range(NCORES)), **kw)
    return _finish(res.results, pred, gt), res


# revision 9
# speedup vs baseline: 6.8599x; 4.2130x over previous
"""PolyMatchingLoss Trainium2 kernel.

Reference computation (B=128, P=1024, C=2):
    dis[b, i] = mean_j sum_c smooth_l1(pred[b,j,c] - gt[b,(i+j)%P,c])
    out = mean_b min_i dis[b, i]

Strategy (v2): two concurrent per-core lanes over the 16 local batches.

  Lane D (DVE, ND batches): one fused custom DVE instruction per
    (b, shift-block) computes 2*smooth_l1(W - P) elementwise over a
    [128 shifts, 2048 (j,c)] tile and sum-reduces along the free axis
    into a [128,1] accumulator column.  smooth_l1 via m*(2t-m), t=|d|,
    m=min(t,1).  The gt operand uses the staircase identity
    W[x, y] = gtflat2[2x+y]; pred is host-replicated to 128 partitions.

  Lane C (ACT+PE, NC batches): uses 2f(d) = d^2 - relu(|d|-1)^2.
    Layout: partition = point-in-block u, free = shift i.
    - Sum_j d^2 = (Sum p^2 + Sum g^2) - 2 corr[i]; the constant is added
      on the host, corr[i] is computed by TensorE as 8 accumulating
      matmuls per c whose stationary operand is the -2*pred block column
      and whose moving operand is the SAME gt window tile the ACT passes
      read.
    - The correction Sum_j relu(|d|-1)^2 runs on ScalarE in 3 passes
      (Abs with per-partition -pred bias; one big Relu(x-1); one big
      Square) and is reduced over partitions by TensorE with a -1s
      stationary column.
    - All matmuls of all NC batches accumulate into one stacked PSUM
      pair [8, 512]x2 via one-hot stationary columns (col = local batch
      index), so PSUM is copied out exactly once per rep.

  min over shifts + mean over batch on host (tiny).
"""

from operator import add as _operator_add

import numpy as np

from concourse import mybir
from concourse import bass, bass_utils
from concourse.tile import TileContext
import concourse.dve_ops as _dve_ops
from concourse.dve_ops import DveOp
from concourse.dve_spec import Spec, Src0, Src1, Zero, One, maxx, minn

# ---------------------------------------------------------------------------
# Workaround: this toolchain's walrus allows at most ONE sync wait per
# instruction; Tile emits 2+.  Split extras onto EventSemaphore carrier
# instructions inserted just before the offending instruction.
# ---------------------------------------------------------------------------
def _split_multi_waits(nc) -> int:
    n = 0
    for fn in nc.m.functions:
        for bb in fn.blocks:
            out = []
            for inst in bb.instructions:
                si = inst.sync_info
                if si is not None and si.on_wait and len(si.on_wait) > 1:
                    for k, w in enumerate(si.on_wait[:-1]):
                        out.append(
                            mybir.InstEventSemaphore(
                                name=f"{inst.name}_wsplit{k}",
                                opcode="EventSemaphore",
                                engine=inst.engine,
                                ins=[],
                                outs=[],
                                sync_info=mybir.SyncInfo(on_wait=[w], on_update=[]),
                            )
                        )
                        n += 1
                    si.on_wait = [si.on_wait[-1]]
                out.append(inst)
            bb.instructions = out
    return n


B = 128
PNUM = 1024
C = 2
NCORES = 8
BL = B // NCORES  # batches per core
FD = PNUM * C  # 2048 free elements per lane-D tile
WW = FD + 256 * 7  # 3840 lane-D window width
WB = 1024 + 128 * 7  # 1920 lane-C per-coordinate window width

ND = 11  # lane-D (DVE) batches per core
NC = BL - ND  # lane-C (ACT+PE) batches per core


# --------------------------------------------------------------------------
# Custom DVE op: out = m*(2t - m) with t=|in0-in1|, m=min(t,1)  (= 2*huber)
#                accum_out = sum over free axis
# --------------------------------------------------------------------------
def _huber_ref(in0, in1, s0, s1, imm2):
    dd = in0.astype(np.float32) - in1.astype(np.float32)
    tt = np.abs(dd)
    mm = np.minimum(tt, 1.0)
    bb = (mm * (2.0 * tt - mm)).astype(np.float32)
    return bb, bb.reshape(bb.shape[0], -1).sum(axis=-1, keepdims=True)


def _make_huber_op() -> DveOp:
    d = Src0 - Src1
    nd = Src1 - Src0
    t = maxx(d, nd)
    m = minn(t, One)
    v = t - m
    w = t + v
    body = m * w
    return DveOp(
        "TENSOR_HUBER2_REDUCE",
        Spec(body=body, accum=_operator_add, accum_init=Zero, reference=_huber_ref),
        subdim=False,
        uops_sha={"v3": "e8f6160a1f1db788", "v4": "8b26f7daea78cb80"},
    )


def _register_op(op: DveOp) -> None:
    if op.name in _dve_ops._SUB_OPCODE_FOR_NAME:
        return
    _dve_ops.OPS.append(op)
    _dve_ops._SUB_OPCODE_FOR_NAME[op.name] = (
        _dve_ops._CUSTOM_DVE_ROW_BASE + len(_dve_ops.OPS) - 1
    )
    _dve_ops.CUSTOM_DVE_SPECS[op.name] = op.spec
    assert _dve_ops._SUB_OPCODE_FOR_NAME[op.name] < 0x20


HUBER_OP = _make_huber_op()
_register_op(HUBER_OP)


# --------------------------------------------------------------------------
# Bass program (SPMD, one program for all 8 cores)
# --------------------------------------------------------------------------
_dt = mybir.dt
_program_cache = {}


def _build_program(reps: int = 1):
    nc = bass.Bass()
    AF = mybir.ActivationFunctionType

    # lane D inputs
    gtw = nc.declare_dram_parameter("gtw", [ND, 2 * FD], _dt.float32, isOutput=False)
    prep = nc.declare_dram_parameter(
        "prep", [ND, 128, FD], _dt.float32, isOutput=False
    )
    # lane C inputs
    gtsepb = nc.declare_dram_parameter(
        "gtsepb", [NC, 2, 2048], _dt.bfloat16, isOutput=False
    )
    pcolc = nc.declare_dram_parameter(
        "pcolc", [NC, 2, 128, 8], _dt.float32, isOutput=False
    )
    statp = nc.declare_dram_parameter(
        "statp", [128, NC * 16 * 8], _dt.bfloat16, isOutput=False
    )
    stato = nc.declare_dram_parameter(
        "stato", [128, NC * 8], _dt.bfloat16, isOutput=False
    )
    # outputs
    acc_out = nc.declare_dram_parameter(
        "acc", [128, ND * 8], _dt.float32, isOutput=True
    )
    accc_out = nc.declare_dram_parameter("accc", [8, 1024], _dt.float32, isOutput=True)

    with TileContext(nc) as tc:
        with (
            tc.tile_pool(name="w", bufs=2) as wpool,
            tc.tile_pool(name="p", bufs=2) as ppool,
            tc.tile_pool(name="s", bufs=2) as spool,
            tc.tile_pool(name="a", bufs=1) as apool,
            tc.tile_pool(name="wb", bufs=2) as wbpool,
            tc.tile_pool(name="pc", bufs=2) as pcpool,
            tc.tile_pool(name="act", bufs=2) as actpool,
            tc.tile_pool(name="st", bufs=1) as stpool,
            tc.tile_pool(name="ac2", bufs=1) as ac2pool,
            tc.tile_pool(name="ps", bufs=1, space="PSUM") as pspool,
        ):
            acc = apool.tile([128, ND * 8], _dt.float32)
            nc.vector.memset(acc[:], 0.0)
            statpt = stpool.tile([128, NC * 16 * 8], _dt.bfloat16, tag="statpt")
            nc.sync.dma_start(out=statpt[:], in_=statp[:])
            statot = stpool.tile([128, NC * 8], _dt.bfloat16, tag="statot")
            nc.sync.dma_start(out=statot[:], in_=stato[:])
            onen = stpool.tile([128, 1], _dt.float32, tag="onen")
            nc.vector.memset(onen[:], -1.0)

            for _rep in range(reps):
                psA = pspool.tile([8, 512], _dt.float32, tag="psA")
                psB = pspool.tile([8, 512], _dt.float32, tag="psB")

                # ---------------- lane C (ACT + PE) ----------------
                # start/stop are per PSUM bank: the first matmul into EACH
                # of psA/psB must carry start=True (clears the bank), else
                # a second execution of the NEFF accumulates onto stale
                # PSUM contents.
                nmm = 0  # per-bank matmul counter (A and B advance together)
                NMM_TOTAL = NC * 2 * 8 * 2
                for bi in range(NC):
                    wb = wbpool.tile([128, 2 * WB], _dt.bfloat16)
                    for c in (0, 1):
                        # staircase: row u = gtsepb[bi, c, u : u + WB]
                        nc.scalar.dma_start(
                            out=wb[:, c * WB : (c + 1) * WB],
                            in_=bass.AP(
                                gtsepb, (bi * 2 + c) * 2048, [[1, 128], [1, WB]]
                            ),
                        )
                    pcol = pcpool.tile([128, 16], _dt.float32)
                    for c in (0, 1):
                        nc.scalar.dma_start(
                            out=pcol[:, c * 8 : (c + 1) * 8], in_=pcolc[bi, c]
                        )
                    for c in (0, 1):
                        # corr matmuls: -2*corr into psum rows [bi]
                        for q in range(8):
                            scol = ((bi * 2 + c) * 8 + q) * 8
                            for h, ps in ((0, psA), (1, psB)):
                                nc.tensor.matmul(
                                    ps[:, :],
                                    statpt[:, scol : scol + 8],
                                    wb[:, c * WB + 128 * q + 512 * h :][:, 0:512],
                                    start=(nmm == 0),
                                    stop=(nmm == NMM_TOTAL - 1),
                                )
                            nmm += 1
                        # ACT passes
                        tt = actpool.tile([128, 8192], _dt.bfloat16, tag="tt")
                        for q in range(8):
                            nc.scalar.activation(
                                tt[:, 1024 * q : 1024 * (q + 1)],
                                wb[:, c * WB + 128 * q :][:, 0:1024],
                                AF.Abs,
                                bias=pcol[:, c * 8 + q : c * 8 + q + 1],
                            )
                        rr = actpool.tile([128, 8192], _dt.bfloat16, tag="rr")
                        nc.scalar.activation(rr[:], tt[:], AF.Relu, bias=onen[:, 0:1])
                        r2 = actpool.tile([128, 8192], _dt.bfloat16, tag="r2")
                        nc.scalar.activation(r2[:], rr[:], AF.Square)
                        # reduction matmuls: -sum_u r2 into psum rows [bi]
                        for q in range(8):
                            for h, ps in ((0, psA), (1, psB)):
                                nc.tensor.matmul(
                                    ps[:, :],
                                    statot[:, bi * 8 : bi * 8 + 8],
                                    r2[:, 1024 * q + 512 * h :][:, 0:512],
                                    start=(nmm == 0),
                                    stop=(nmm == NMM_TOTAL - 1),
                                )
                            nmm += 1

                # ---------------- lane D (DVE) ----------------
                for b in range(ND):
                    w = wpool.tile([128, WW], _dt.float32)
                    # staircase window: row x = gtflat2[b, 2x : 2x + WW]
                    nc.sync.dma_start(
                        out=w[:], in_=bass.AP(gtw, b * 2 * FD, [[2, 128], [1, WW]])
                    )
                    p = ppool.tile([128, FD], _dt.float32)
                    nc.sync.dma_start(out=p[:], in_=prep[b])
                    for qi in range(8):
                        scr = spool.tile([128, FD], _dt.float32)
                        col = b * 8 + qi
                        nc.vector._custom_dve(
                            HUBER_OP,
                            out=scr[:],
                            in0=w[:, 256 * qi : 256 * qi + FD],
                            in1=p[:],
                            accum_out=acc[:, col : col + 1],
                        )

                # psum -> sbuf -> dram for lane C
                accc = ac2pool.tile([8, 1024], _dt.float32)
                nc.vector.tensor_copy(accc[:, 0:512], psA[:])
                nc.vector.tensor_copy(accc[:, 512:1024], psB[:])
                nc.scalar.dma_start(out=accc_out[:], in_=accc[:])

            nc.sync.dma_start(out=acc_out[:], in_=acc[:])
    _split_multi_waits(nc)
    # Raw Bass (unlike Bacc.compile) never runs this pass; without it the
    # custom-DVE InstISA subclasses serialize with empty .instr bytes and
    # walrus fails with "ISA wrong length".
    mybir.codegen_inst_isa_subclasses(nc)
    return nc


def _get_program():
    if "nc" not in _program_cache:
        _program_cache["nc"] = _build_program()
    return _program_cache["nc"]


# --------------------------------------------------------------------------
# Host wrapper
# --------------------------------------------------------------------------
def _make_in_maps(pred: np.ndarray, gt: np.ndarray):
    pred = np.ascontiguousarray(pred, dtype=np.float32)
    gt = np.ascontiguousarray(gt, dtype=np.float32)
    in_maps = []
    for core in range(NCORES):
        sl = slice(core * BL, (core + 1) * BL)
        gtc = gt[sl]  # [BL, P, C]
        predc = pred[sl]  # [BL, P, C]
        gtdupc = np.concatenate([gtc, gtc], axis=1)  # [BL, 2P, C]
        # lane D
        gtdup = gtdupc[:ND].reshape(ND, 2 * FD)
        prepc = np.ascontiguousarray(
            np.broadcast_to(predc[:ND].reshape(ND, 1, FD), (ND, 128, FD))
        )
        # lane C
        gtsepb = np.ascontiguousarray(
            gtdupc[ND:].transpose(0, 2, 1)
        )  # [NC, 2, 2048] f32
        pcolc = np.ascontiguousarray(
            (-predc[ND:]).reshape(NC, 8, 128, 2).transpose(0, 3, 2, 1)
        )  # [NC, 2, 128, 8] = -pred[b, 128q+u, c]
        # stationary tiles
        pblk = predc[ND:].reshape(NC, 8, 128, 2).transpose(2, 0, 3, 1)  # [128,NC,2,8]
        statp = np.zeros((128, NC, 2, 8, 8), dtype=np.float32)
        for bi in range(NC):
            statp[:, bi, :, :, bi] = -2.0 * pblk[:, bi, :, :]
        statp = statp.reshape(128, NC * 16 * 8)
        stato = np.zeros((128, NC, 8), dtype=np.float32)
        for bi in range(NC):
            stato[:, bi, bi] = -1.0
        stato = stato.reshape(128, NC * 8)
        in_maps.append(
            {
                "gtw": gtdup,
                "prep": prepc,
                "gtsepb": _to_bf16(gtsepb),
                "pcolc": pcolc,
                "statp": _to_bf16(statp),
                "stato": _to_bf16(stato),
            }
        )
    return in_maps


def _to_bf16(a: np.ndarray) -> np.ndarray:
    import ml_dtypes

    return a.astype(ml_dtypes.bfloat16)


def _finish(results, pred: np.ndarray, gt: np.ndarray) -> np.float32:
    pred = np.asarray(pred, dtype=np.float64)
    gt = np.asarray(gt, dtype=np.float64)
    mins = []
    for core in range(NCORES):
        sl = slice(core * BL, (core + 1) * BL)
        # lane D
        acc = np.asarray(results[core]["acc"], dtype=np.float32)  # [128, ND*8]
        acc = acc.reshape(128, ND, 8)  # [i_local, b, qi]
        dis = acc.transpose(1, 2, 0).reshape(ND, PNUM) / (2.0 * PNUM)
        mins.append(dis.min(axis=1))
        # lane C: 2P*dis = qc + psum  (psum = -2corr - sum rsq)
        accc = np.asarray(results[core]["accc"], dtype=np.float64)[:NC]  # [NC,1024]
        pc = pred[sl][ND:]
        gc = gt[sl][ND:]
        qc = (pc * pc).sum(axis=(1, 2)) + (gc * gc).sum(axis=(1, 2))  # [NC]
        disc = (qc[:, None] + accc) / (2.0 * PNUM)
        mins.append(disc.min(axis=1).astype(np.float32))
    return np.asarray(np.mean(np.concatenate(mins)), dtype=np.float32)


def kernel(pred: np.ndarray, gt: np.ndarray) -> np.ndarray:
    nc = _get_program()
    in_maps = _make_in_maps(pred, gt)
    res = bass_utils.run_bass_kernel_spmd(nc, in_maps, list(range(NCORES)))
    return _finish(res.results, pred, gt)


# Exposed for test.py: run with tracing and return (value, BassKernelResults)
def kernel_traced(pred: np.ndarray, gt: np.ndarray, **kw):
    nc = _get_program()
    in_maps = _make_in_maps(pred, gt)
    res = bass_utils.run_bass_kernel_spmd(nc, in_maps, list(range(NCORES)), **kw)
    return _finish(res.results, pred, gt), res
